# revision 1
# baseline (speedup 1.0000x reference)
"""CustomLSTM Trainium2 kernel.

Problem: x [64, 1024, 256], LSTM(I=256, H=512), output h_last @ fc_w.T + fc_b -> [64, 1].

Strategy (data-parallel over batch, 8 cores x 8 sequences):
- Per core, precompute xproj = x @ U_cat + b_cat in chunks of TC=16 timesteps
  (f32r matmuls, PE-transposing x tiles on the fly), kept in SBUF.
- Recurrence per step t:
    psum1[8, 2048]  = h_{t-1} @ V_cat     (16 f32r matmuls, h.T stationary)
    gates[8, 2048]  = psum1 + xproj[t]    (4 DVE adds, fused copy to SBUF)
    psum2[128, 128] = gates.T             (16 PE transposes, gate-chunk-major)
    i,f,o = sigmoid, g = tanh             (ACT, [128, 32] tiles)
    c = f*c + i*g; h.T = o * tanh(c)      (DVE, [128, 32] tiles)
- Final: y = h_last @ fc_w.T + fc_b via 4 tiny matmuls -> [1, 8] per core.

Gate column order is host-permuted to [i, f, o, g]; the g-chunk is computed
first each step so the c-chain overlaps the remaining matmuls.
"""
import sys

if "/opt/trn_rl_repo" not in sys.path:
    sys.path.insert(0, "/opt/trn_rl_repo")

import numpy as np
from contextlib import ExitStack

import concourse.bass as bass
import concourse.bacc as bacc
import concourse.tile as tile
import concourse.mybir as mybir
from concourse.bass_utils import run_bass_kernel_spmd

F32 = mybir.dt.float32
F32R = mybir.dt.float32r
AF = mybir.ActivationFunctionType

B, T, I, H = 64, 1024, 256, 512
NCORES = 8
BC = B // NCORES           # 8 sequences per core
G4 = 4 * H                 # 2048
TC = 16                    # timesteps per xproj chunk
NCHUNK = T // TC
# n-chunk processing order: g first (c-chain starts early), o last
NORDER = (3, 0, 1, 2)


def build_program(n_steps=T):
    nc = bacc.Bacc("TRN2", target_bir_lowering=False, debug=False,
                   num_devices=NCORES)

    xc = nc.dram_tensor("xc", [BC, T, I], F32, kind="ExternalInput")
    Vc = nc.dram_tensor("Vc", [H, G4], F32, kind="ExternalInput")
    Uc = nc.dram_tensor("Uc", [I, G4], F32, kind="ExternalInput")
    bc = nc.dram_tensor("bc", [1, G4], F32, kind="ExternalInput")
    eye8 = nc.dram_tensor("eye8", [8, 8], F32, kind="ExternalInput")
    eye128 = nc.dram_tensor("eye128", [128, 128], F32, kind="ExternalInput")
    ones1 = nc.dram_tensor("ones1", [1, 128], F32, kind="ExternalInput")
    fcw = nc.dram_tensor("fcw", [128, 4], F32, kind="ExternalInput")
    fcb = nc.dram_tensor("fcb", [1, 8], F32, kind="ExternalInput")
    y8 = nc.dram_tensor("y8", [1, BC], F32, kind="ExternalOutput")

    n_chunks = (n_steps + TC - 1) // TC

    with ExitStack() as ctx:
        tc_ = ctx.enter_context(tile.TileContext(nc))

        consts = ctx.enter_context(tc_.tile_pool(name="consts", bufs=1))
        xpool = ctx.enter_context(tc_.tile_pool(name="xpool", bufs=2))
        xproj_pool = ctx.enter_context(tc_.tile_pool(name="xproj", bufs=2))
        stage_pool = ctx.enter_context(tc_.tile_pool(name="stage", bufs=4))
        state_pool = ctx.enter_context(tc_.tile_pool(name="state", bufs=3))
        work_pool = ctx.enter_context(tc_.tile_pool(name="work", bufs=2))

        ps_g = ctx.enter_context(tc_.tile_pool(name="ps_g", bufs=1, space="PSUM"))
        ps_t = ctx.enter_context(tc_.tile_pool(name="ps_t", bufs=2, space="PSUM"))
        ps_x = ctx.enter_context(tc_.tile_pool(name="ps_x", bufs=1, space="PSUM"))

        # ---- constants: DMA to staging, DVE relay so matmuls have 1-sem deps
        def relay(dram_ap, shape, dtype, tag):
            st = consts.tile(shape, dtype, tag=f"{tag}_st")
            nc.sync.dma_start(st[:], dram_ap)
            dst = consts.tile(shape, dtype, tag=tag)
            nc.vector.tensor_copy(dst[:], st[:])
            return dst

        V_sb = [relay(Vc[k * 128:(k + 1) * 128, :].bitcast(F32R), [128, G4], F32R,
                      f"V{k}") for k in range(4)]
        U_sb = [relay(Uc[k * 128:(k + 1) * 128, :].bitcast(F32R), [128, G4], F32R,
                      f"U{k}") for k in range(2)]
        bc_sb = relay(bc[:].bitcast(F32R), [1, G4], F32R, "bc")
        eye8_sb = relay(eye8[:].bitcast(F32R), [8, 8], F32R, "eye8")
        eye128_sb = relay(eye128[:].bitcast(F32R), [128, 128], F32R, "eye128")
        ones_sb = relay(ones1[:].bitcast(F32R), [1, 128], F32R, "ones")
        fcw_sb = relay(fcw[:].bitcast(F32R), [128, 4], F32R, "fcw")
        fcb_sb = relay(fcb[:], [1, 8], F32, "fcb")

        # ---- initial state h=0, c=0
        c_t = state_pool.tile([128, 4 * BC], F32, tag="c")
        nc.vector.memset(c_t[:], 0.0)
        hT = state_pool.tile([128, 4 * BC], F32R, tag="hT")
        nc.vector.tensor_copy(hT[:], c_t[:])

        xproj_chunks = [None] * n_chunks

        def emit_xproj_chunk(ci):
            """xproj for steps [ci*TC, ci*TC+TC) -> sbuf tile [128, G4]
            (partition = t_local*8 + b, free = gate col)."""
            x_t = xpool.tile([128, I], F32R, tag="x")
            nc.sync.dma_start(
                x_t[:],
                xc[:, ci * TC:(ci + 1) * TC, :].rearrange(
                    "b t i -> t b i").bitcast(F32R),
            )
            xT = []
            for k in range(2):
                pX = ps_x.tile([128, 128], F32R, tag="psx")
                nc.tensor.transpose(pX[:], x_t[:, k * 128:(k + 1) * 128],
                                    eye128_sb[:])
                xk = xpool.tile([128, 128], F32R, tag=f"xT{k}")
                nc.vector.tensor_copy(xk[:], pX[:])
                xT.append(xk)
            xp = xproj_pool.tile([128, G4], F32, tag="xproj")
            for n in range(4):
                pM = ps_x.tile([128, 512], F32, tag="psmm")
                nc.tensor.matmul(pM[:], xT[0][:], U_sb[0][:, n * 512:(n + 1) * 512],
                                 start=True, stop=False)
                nc.tensor.matmul(pM[:], xT[1][:], U_sb[1][:, n * 512:(n + 1) * 512],
                                 start=False, stop=False)
                nc.tensor.matmul(pM[:], ones_sb[:], bc_sb[:, n * 512:(n + 1) * 512],
                                 start=False, stop=True)
                nc.vector.tensor_copy(xp[:, n * 512:(n + 1) * 512], pM[:])
            xproj_chunks[ci] = xp

        # staging prefetch: per-step [8, G4] slice at partition base 0
        stages = {}

        def emit_stage(t):
            if t >= n_steps:
                return
            ci, tl = t // TC, t % TC
            st = stage_pool.tile([BC, G4], F32, tag="xstage")
            nc.sync.dma_start(st[:], xproj_chunks[ci][tl * BC:(tl + 1) * BC, :])
            stages[t] = st

        emit_xproj_chunk(0)
        if n_chunks > 1:
            emit_xproj_chunk(1)
        emit_stage(0)
        emit_stage(1)

        for t in range(n_steps):
            st = stages.pop(t)
            psum1 = ps_g.tile([BC, G4], F32, tag="psum1")
            gates = work_pool.tile([BC, G4], F32R, tag="gates")
            psum2 = ps_t.tile([128, 4 * BC * 4], F32R, tag="psum2")
            acts = {}
            new_hT = state_pool.tile([128, 4 * BC], F32R, tag="hT")
            new_c = state_pool.tile([128, 4 * BC], F32, tag="c")

            for n in NORDER:
                sl = slice(n * 512, (n + 1) * 512)
                for k in range(4):
                    nc.tensor.matmul(
                        psum1[:, sl], hT[:, k * BC:(k + 1) * BC],
                        V_sb[k][:, sl], start=(k == 0), stop=(k == 3))
                nc.vector.tensor_add(gates[:, sl], psum1[:, sl], st[:, sl])
                for j4 in range(4):
                    j = 4 * n + j4
                    nc.tensor.transpose(
                        psum2[:, j * BC:(j + 1) * BC],
                        gates[:, j * 128:(j + 1) * 128], eye8_sb[:])
                gsl = slice(n * 4 * BC, (n + 1) * 4 * BC)
                gT = work_pool.tile([128, 4 * BC], F32, tag=f"gT{n}")
                nc.vector.tensor_copy(gT[:], psum2[:, gsl])
                ga = work_pool.tile([128, 4 * BC], F32, tag=f"act{n}")
                nc.scalar.activation(ga[:], gT[:],
                                     AF.Tanh if n == 3 else AF.Sigmoid)
                acts[n] = ga
                if n == 0:  # i ready (g already done): ig = i * g
                    ig = work_pool.tile([128, 4 * BC], F32, tag="ig")
                    nc.vector.tensor_mul(ig[:], acts[0][:], acts[3][:])
                elif n == 1:  # f ready: fc = f * c; c_new = fc + ig
                    fcx = work_pool.tile([128, 4 * BC], F32, tag="fcx")
                    nc.vector.tensor_mul(fcx[:], acts[1][:], c_t[:])
                    nc.vector.tensor_add(new_c[:], ig[:], fcx[:])
                    tc_tile = work_pool.tile([128, 4 * BC], F32, tag="tanhc")
                    nc.scalar.activation(tc_tile[:], new_c[:], AF.Tanh)
                elif n == 2:  # o ready: h.T = o * tanh(c)
                    nc.vector.tensor_mul(new_hT[:], acts[2][:], tc_tile[:])

            hT, c_t = new_hT, new_c

            # prefetch pipeline
            if t % TC == 6:
                ci = t // TC + 2
                if ci < n_chunks:
                    emit_xproj_chunk(ci)
            emit_stage(t + 2)

        # ---- final FC: y = h.T-reduced @ fc_w + fc_b
        ps_fc = ps_x.tile([1, BC], F32, tag="psmm")
        for k in range(4):
            nc.tensor.matmul(ps_fc[:], fcw_sb[:, k:k + 1],
                             hT[:, k * BC:(k + 1) * BC],
                             start=(k == 0), stop=(k == 3))
        y_sb = consts.tile([1, BC], F32, tag="y")
        nc.vector.tensor_add(y_sb[:], ps_fc[:], fcb_sb[:])
        nc.sync.dma_start(y8[:], y_sb[:])

    nc.compile()
    return nc


def prep_inputs(x, U_i, V_i, b_i, U_f, V_f, b_f, U_h, V_h, b_h, U_o, V_o, b_o,
                fc_w, fc_b):
    # gate order [i, f, o, g]; g == reference's "h" gate
    U_cat = np.ascontiguousarray(
        np.concatenate([U_i, U_f, U_o, U_h], axis=1), dtype=np.float32)
    V_cat = np.ascontiguousarray(
        np.concatenate([V_i, V_f, V_o, V_h], axis=1), dtype=np.float32)
    b_cat = np.ascontiguousarray(
        np.concatenate([b_i, b_f, b_o, b_h])[None, :], dtype=np.float32)
    fcw = np.ascontiguousarray(
        np.asarray(fc_w, dtype=np.float32).reshape(4, 128).T)
    fcb = np.full((1, 8), float(np.asarray(fc_b).reshape(-1)[0]), np.float32)
    shared = {
        "Vc": V_cat, "Uc": U_cat, "bc": b_cat,
        "eye8": np.eye(8, dtype=np.float32),
        "eye128": np.eye(128, dtype=np.float32),
        "ones1": np.ones((1, 128), np.float32),
        "fcw": fcw, "fcb": fcb,
    }
    x = np.ascontiguousarray(np.asarray(x, dtype=np.float32))
    in_maps = []
    for c in range(NCORES):
        m = dict(shared)
        m["xc"] = np.ascontiguousarray(x[c * BC:(c + 1) * BC])
        in_maps.append(m)
    return in_maps


_CACHED = {}


def kernel(**inputs) -> np.ndarray:
    in_maps = prep_inputs(**inputs)
    if "nc" not in _CACHED:
        _CACHED["nc"] = build_program()
    nc = _CACHED["nc"]
    res = run_bass_kernel_spmd(nc, in_maps, core_ids=list(range(NCORES)))
    _CACHED["last_results"] = res
    out = np.empty((B, 1), np.float32)
    for c in range(NCORES):
        out[c * BC:(c + 1) * BC, 0] = res.results[c]["y8"][0]
    return out


if __name__ == "__main__":
    import reference

    inputs = {k: np.asarray(v) for k, v in reference.setup_inputs().items()}
    exp = np.asarray(reference.reference(**inputs))
    got = kernel(**inputs)
    err = np.abs(got - exp).max()
    rel = np.linalg.norm(got - exp) / np.linalg.norm(exp)
    print(f"max abs err: {err:.3e}  rel err: {rel:.3e}")

